# revision 28
# baseline (speedup 1.0000x reference)
"""Trainium2 Bass kernel for nn_CrossAttention_84310208020733.

Cross-attention: out = proj(softmax(mask(q @ k^T * scale)) @ v), with
  q = tgt @ q_w.T + q_b               [B=4, NT=1024, D=1024]
  k, v = split(src @ kv_w.T + kv_b)   [B=4, NS=2048, D=1024], H=16 heads, Dh=64

Sharding over 8 NeuronCores: core c handles batch b = c//2 and head group
g = c%2 (8 heads = 512 channels).  Each core computes its partial
proj-output (contraction over its 512 attn channels) in transposed layout
[out_ch, rows]; the host sums the two partials per batch, transposes, and
adds proj_b (the "all-reduce after proj" done at gather time).

On-device layout is feature-major throughout ("T" = channels on SBUF
partitions):
  qT = qwT.T @ tgtT       [512, 1024]
  kT = kwT.T @ srcT       [512, NS_kept]
  v  = srcT.T @ vwT       [NS_kept, 512]   (+ ones/zero columns for row-sums)
  sT = kT_h.T @ qT_h      [src 128, rows 512] per head pair (row-packed K=64)
  pT = exp(sT * scale + maskbias)  (ACT, bf16 out; no max-subtraction: |logits|<~4)
  av = [v_h | 1].T @ pT   -> [Dh(+1), rows] unnormalized out + row sums
  oT = av * bcast(1/sum)  [512, 1024]   (recip on DVE, bcast on GpSimd)
  outT = pwT.T @ oT       [1024, 1024] partial, fp32

Fully-masked 128-wide src chunks (per the runtime mask, intersected across
batches) are dropped at compile time; partial masks are handled via the
additive -30000 bias inside the exp activation.

Scheduling: input DMAs are split into ~70 medium descriptors issued
round-robin from the three DGE-capable engines (SP, ACT, GpSimd) in
consumption order, so loads stream at full HBM rate from ~6us.  The PE
stream is one dense sequence: q-projection, k-projection(t=0), then 8
attention blocks (t-pair x row-half) with the remaining k/v/q projections
and the output projection woven in as fillers between score/AV matmuls.
Each block's normalization is emitted at the start of the NEXT block so
the PE never waits on it; row-sum reciprocals broadcast across partitions
on GpSimd instead of PE matmuls.  Output tiles stream out per projection
group on two DGE engines.
"""

import numpy as np
import ml_dtypes

import concourse.bass as bass
import concourse.bacc as bacc
import concourse.tile as tile
from concourse import mybir
from concourse.bass_utils import run_bass_kernel_spmd

P = 128
B = 4
NT = 1024
NS = 2048
D = 1024
H = 16
DH = 64
G = 2              # head groups (tensor-parallel dim)
HG = H // G        # heads per core = 8
CH = HG * DH       # channels per core = 512
KO = D // P        # 8 contraction chunks for the projections
CHO = CH // P      # 4 channel tiles per core
SCALE = DH ** -0.5
NEG = -30000.0
BF16 = mybir.dt.bfloat16
F32 = mybir.dt.float32
EXP = mybir.ActivationFunctionType.Exp

# vaug per-pair block: [A: 64 ch + 1 ones][B: 1 ones + 63 zero + 64 ch]
ABLK = DH + 1            # 65
BBLK = P                 # 128
PBLK = ABLK + BBLK       # 193


def _build_nc(nk: int) -> "bacc.Bacc":
    """Emit the per-core program for nk kept 128-wide source chunks."""
    ns_k = nk * P
    NSB = ns_k // 512
    nc = bacc.Bacc("TRN2", target_bir_lowering=False, debug=False)

    tgtT = nc.dram_tensor("tgtT", [D, NT], BF16, kind="ExternalInput")
    srcT = nc.dram_tensor("srcT", [D, ns_k], BF16, kind="ExternalInput")
    qwT = nc.dram_tensor("qwT", [D, CH], BF16, kind="ExternalInput")
    kwT = nc.dram_tensor("kwT", [D, CH], BF16, kind="ExternalInput")
    vwT = nc.dram_tensor("vwT", [D, CH], BF16, kind="ExternalInput")
    pwT = nc.dram_tensor("pwT", [CH, D], BF16, kind="ExternalInput")
    qb = nc.dram_tensor("qb", [CH], F32, kind="ExternalInput")
    kb = nc.dram_tensor("kb", [CH], F32, kind="ExternalInput")
    vb = nc.dram_tensor("vb", [CH], F32, kind="ExternalInput")
    maskT = nc.dram_tensor("maskT", [P, nk], F32, kind="ExternalInput")
    outT = nc.dram_tensor("outT", [D, NT], BF16, kind="ExternalOutput")

    with tile.TileContext(nc) as tc:
        with (
            tc.tile_pool(name="persist", bufs=1) as pers,
            tc.tile_pool(name="work", bufs=3) as work,
            tc.tile_pool(name="ps", bufs=2, space="PSUM") as ps,
        ):
            # ---- persistent tiles --------------------------------------
            w_q = [pers.tile([P, CH], BF16, tag=f"w_q{k}", name=f"w_q{k}")
                   for k in range(KO)]
            tgt_t = [pers.tile([P, NT], BF16, tag=f"tgt{k}", name=f"tgt{k}")
                     for k in range(KO)]
            w_k = [pers.tile([P, CH], BF16, tag=f"w_k{k}", name=f"w_k{k}")
                   for k in range(KO)]
            src_t = [pers.tile([P, ns_k], BF16, tag=f"src{k}", name=f"src{k}")
                     for k in range(KO)]
            w_v = [pers.tile([P, CH], BF16, tag=f"w_v{k}", name=f"w_v{k}")
                   for k in range(KO)]
            w_p = pers.tile([P, CHO, D], BF16, tag="w_p")
            mask_t = pers.tile([P, nk], F32, tag="mask_t")
            qb_t = pers.tile([P, CHO], F32, tag="qb_t")
            kb_t = pers.tile([P, CHO], F32, tag="kb_t")
            vb_bc = pers.tile([P, CH], F32, tag="vb_bc")
            ones_t = pers.tile([P, P], BF16, tag="ones_t")
            nc.vector.memset(ones_t[:], 1.0)
            qT = pers.tile([P, CHO, NT], BF16, tag="qT")
            kT = pers.tile([P, CHO, ns_k], BF16, tag="kT")
            oT = pers.tile([P, CHO, NT], BF16, tag="oT")
            vaug = [pers.tile([P, HG // 2 * PBLK], BF16, tag=f"vaug{i}",
                              name=f"vaug{i}")
                    for i in range(nk)]

            # ---- input DMA issue: 3 engines, consumption order ---------
            nc.gpsimd.dma_start(out=mask_t[:], in_=maskT.ap())
            nc.gpsimd.dma_start(out=qb_t[:],
                                in_=qb.ap().rearrange("(o p) -> p o", p=P))
            nc.gpsimd.dma_start(out=kb_t[:],
                                in_=kb.ap().rearrange("(o p) -> p o", p=P))

            descs = []
            # first chunk split in half so the very first matmul starts early
            descs.append((w_q[0][:, 0:256], qwT.ap()[0:P, 0:256]))
            descs.append((tgt_t[0][:, 0:256], tgtT.ap()[0:P, 0:256]))
            descs.append((w_q[0][:, 256:512], qwT.ap()[0:P, 256:512]))
            descs.append((tgt_t[0][:, 256:512], tgtT.ap()[0:P, 256:512]))
            for k in range(1, 4):
                descs.append((w_q[k][:], qwT.ap()[k * P:(k + 1) * P, :]))
                descs.append((tgt_t[k][:, 0:256],
                              tgtT.ap()[k * P:(k + 1) * P, 0:256]))
                descs.append((tgt_t[k][:, 256:512],
                              tgtT.ap()[k * P:(k + 1) * P, 256:512]))
            for k in range(4, KO):
                descs.append((w_q[k][:], qwT.ap()[k * P:(k + 1) * P, :]))
                descs.append((tgt_t[k][:, 0:512],
                              tgtT.ap()[k * P:(k + 1) * P, 0:512]))
            for k in range(KO):
                descs.append((w_k[k][:], kwT.ap()[k * P:(k + 1) * P, :]))
                descs.append((src_t[k][:, 0:512],
                              srcT.ap()[k * P:(k + 1) * P, 0:512]))
            for k in range(KO):
                descs.append((w_v[k][:], vwT.ap()[k * P:(k + 1) * P, :]))
            for x in range(1, NSB):
                for k in range(KO):
                    descs.append((src_t[k][:, x * 512:(x + 1) * 512],
                                  srcT.ap()[k * P:(k + 1) * P,
                                            x * 512:(x + 1) * 512]))
            for k in range(KO):
                descs.append((tgt_t[k][:, 512:1024],
                              tgtT.ap()[k * P:(k + 1) * P, 512:1024]))
            for o in range(CHO):
                descs.append((w_p[:, o, :], pwT.ap()[o * P:(o + 1) * P, :]))
            dma_engs = [nc.sync, nc.scalar, nc.gpsimd]
            eng_lists = [[], [], []]
            for i, dsc in enumerate(descs):
                eng_lists[i % 3].append(dsc)
            # vb broadcast (software-DGE, ~3us) goes mid-queue on gpsimd so
            # it doesn't delay the early weight chunks but lands before the
            # v-projection weave needs it
            vb_ap = vb.ap()
            vb_src = bass.AP(tensor=vb_ap.tensor, offset=vb_ap.offset,
                             ap=[[0, P]] + list(vb_ap.ap))
            eng_lists[2].insert(8, (vb_bc[:], vb_src))
            for eng, lst in zip(dma_engs, eng_lists):
                for o_, i_ in lst:
                    eng.dma_start(out=o_, in_=i_)

            # ---- emission units ----------------------------------------
            def qt_group(m, n):
                pmm = ps.tile([P, 512], F32, tag="acc", bufs=2, name="pmm_q")
                for k in range(KO):
                    nc.tensor.matmul(
                        pmm[:], w_q[k][:, m * P:(m + 1) * P],
                        tgt_t[k][:, n * 512:(n + 1) * 512],
                        start=(k == 0), stop=(k == KO - 1))
                nc.vector.tensor_scalar_add(
                    qT[:, m, n * 512:(n + 1) * 512], pmm[:], qb_t[:, m:m + 1])

            def kt_group(m, x):
                pmm = ps.tile([P, 512], F32, tag="acc", bufs=2, name="pmm_k")
                for k in range(KO):
                    nc.tensor.matmul(
                        pmm[:], w_k[k][:, m * P:(m + 1) * P],
                        src_t[k][:, x * 512:(x + 1) * 512],
                        start=(k == 0), stop=(k == KO - 1))
                nc.vector.tensor_scalar_add(
                    kT[:, m, x * 512:(x + 1) * 512], pmm[:], kb_t[:, m:m + 1])

            def v_group(ms):
                pmm = ps.tile([P, 512], F32, tag="acc", bufs=2, name="pmm_v")
                for k in range(KO):
                    nc.tensor.matmul(
                        pmm[:], src_t[k][:, ms * P:(ms + 1) * P], w_v[k][:],
                        start=(k == 0), stop=(k == KO - 1))
                va = vaug[ms].rearrange("p (t c) -> p t c", c=PBLK)
                pv = pmm.rearrange("p (t c) -> p t c", c=2 * DH)
                vv = vb_bc.rearrange("p (t c) -> p t c", c=2 * DH)
                nc.vector.tensor_add(va[:, :, 0:DH], pv[:, :, 0:DH],
                                     vv[:, :, 0:DH])
                nc.vector.tensor_add(va[:, :, ABLK + DH:PBLK],
                                     pv[:, :, DH:2 * DH], vv[:, :, DH:2 * DH])
                nc.vector.memset(va[:, :, DH:DH + 2], 1.0)
                nc.vector.memset(va[:, :, ABLK + 1:ABLK + DH], 0.0)

            def proj_start(m, n, tag):
                # first 3 contraction chunks only -- runnable before the
                # last block's normalization lands (k=CHO-1 = that block)
                pmm = ps.tile([P, 512], F32, tag=tag, bufs=2, name="pmm_p")
                for k in range(CHO - 1):
                    nc.tensor.matmul(
                        pmm[:], w_p[:, k, m * P:(m + 1) * P],
                        oT[:, k, n * 512:(n + 1) * 512],
                        start=(k == 0), stop=False)
                return pmm

            def proj_finish(m, n, pmm, use_scalar=False):
                k = CHO - 1
                nc.tensor.matmul(
                    pmm[:], w_p[:, k, m * P:(m + 1) * P],
                    oT[:, k, n * 512:(n + 1) * 512],
                    start=False, stop=True)
                _proj_store(m, n, pmm, use_scalar)

            def _proj_store(m, n, pmm, use_scalar):
                ob = work.tile([P, 512], BF16, tag="ob", bufs=4, name="ob")
                # both half-copies first (freeing the PSUM bank asap), then
                # the stores; tail groups split copies across DVE+ACT and
                # store via the two hardware DGEs (the software DGE on
                # GpSimd has a multi-us drain at kernel end)
                if use_scalar:
                    nc.vector.tensor_copy(ob[:, 0:256], pmm[:, 0:256])
                    nc.scalar.copy(ob[:, 256:512], pmm[:, 256:512])
                    engs = (nc.sync, nc.scalar)
                else:
                    nc.vector.tensor_copy(ob[:], pmm[:])
                    engs = (nc.sync, nc.gpsimd)
                for h, eng in enumerate(engs):
                    eng.dma_start(
                        out=outT.ap()[m * P:(m + 1) * P,
                                      n * 512 + h * 256:n * 512 + (h + 1) * 256],
                        in_=ob[:, h * 256:(h + 1) * 256])

            def proj_group(m, n, use_scalar=False):
                pmm = ps.tile([P, 512], F32, tag="acc", bufs=2, name="pmm_p")
                for k in range(CHO):
                    nc.tensor.matmul(
                        pmm[:], w_p[:, k, m * P:(m + 1) * P],
                        oT[:, k, n * 512:(n + 1) * 512],
                        start=(k == 0), stop=(k == CHO - 1))
                _proj_store(m, n, pmm, use_scalar)

            def _norm1(t, n, avA, avB, st8):
                # stage 1 (at j0 of next block): drain av accumulators to
                # SBUF, freeing the PSUM bank pair for the next block's avs
                avAs = work.tile([ABLK, 512], BF16, tag="avAs", bufs=2,
                                 name="avAs")
                nc.vector.tensor_copy(avAs[:], avA[:])
                avBs = work.tile([P, 512], BF16, tag="avBs", bufs=2,
                                 name="avBs")
                nc.vector.tensor_copy(avBs[:], avB[:])
                st8["avAs"], st8["avBs"] = avAs, avBs

            def _norm2(t, n, st8):
                # stage 2 (at j2): PE broadcast of the row sums, reciprocal,
                # scale -- by now the stage-1 copies have long retired
                rsl = slice(n * 512, (n + 1) * 512)
                avAs, avBs = st8["avAs"], st8["avBs"]
                bsA = ps.tile([P, 512], F32, tag="acc", bufs=2, name="bsA")
                nc.tensor.matmul(bsA[:], ones_t[DH:DH + 1, :],
                                 avAs[DH:DH + 1, :], start=True, stop=True)
                rbA = work.tile([P, 512], F32, tag="rbA", bufs=2, name="rbA")
                nc.vector.reciprocal_approx_fast(rbA[:], bsA[:])
                nc.vector.tensor_mul(oT[0:DH, t, rsl], avAs[0:DH, :],
                                     rbA[0:DH, :])
                bsB = ps.tile([P, 512], F32, tag="acc", bufs=2, name="bsB")
                nc.tensor.matmul(bsB[:], ones_t[0:1, :], avBs[0:1, :],
                                 start=True, stop=True)
                rbB = work.tile([P, 512], F32, tag="rbB", bufs=2, name="rbB")
                nc.vector.reciprocal_approx_fast(rbB[:], bsB[:])
                nc.vector.tensor_mul(oT[DH:P, t, rsl], avBs[DH:P, :],
                                     rbB[DH:P, :])

            def av_pair(j, pt, avA, avB, t):
                va = vaug[j].rearrange("p (t c) -> p t c", c=PBLK)
                nc.tensor.matmul(avA[:], va[:, t, 0:ABLK], pt[:, 0:512],
                                 start=(j == 0), stop=(j == nk - 1))
                nc.tensor.matmul(avB[:], va[:, t, ABLK:PBLK],
                                 pt[:, 512:1024],
                                 start=(j == 0), stop=(j == nk - 1))

            def attn_block(t, n, fillers=(), prev_norm=None, pre_chunk=None):
                rsl = slice(n * 512, (n + 1) * 512)
                avA = ps.tile([ABLK, 512], F32, tag="av", bufs=2, name="avA")
                avB = ps.tile([P, 512], F32, tag="av", bufs=2, name="avB")
                fq = list(fillers)
                pts = []
                for j in range(nk):
                    if pre_chunk is not None:
                        pre_chunk(j)
                    st = ps.tile([P, 1024], F32, tag="st", bufs=2, name="st")
                    nc.tensor.matmul(
                        st[:, 0:512], kT[0:DH, t, j * P:(j + 1) * P],
                        qT[0:DH, t, rsl], start=True, stop=True,
                        tile_position=(0, 0))
                    nc.tensor.matmul(
                        st[:, 512:1024], kT[DH:P, t, j * P:(j + 1) * P],
                        qT[DH:P, t, rsl], start=True, stop=True,
                        tile_position=(64, 0))
                    pt = work.tile([P, 1024], BF16, tag="pt", bufs=6,
                                   name="pt")
                    nc.scalar.activation(out=pt[:], in_=st[:], func=EXP,
                                         bias=mask_t[:, j:j + 1], scale=SCALE)
                    pts.append(pt)
                    if j == 0 and prev_norm is not None:
                        prev_norm[0]()
                    if j == 2 and prev_norm is not None:
                        prev_norm[1]()
                    if j >= 1:
                        av_pair(j - 1, pts[j - 1], avA, avB, t)
                    if fq and j % 2 == 1:
                        fq.pop(0)()
                av_pair(nk - 1, pts[nk - 1], avA, avB, t)
                while fq:
                    fq.pop(0)()
                st8 = {}
                return (lambda: _norm1(t, n, avA, avB, st8),
                        lambda: _norm2(t, n, st8))

            # ---- schedule ----------------------------------------------
            def mk(f, *a):
                return lambda: f(*a)

            # interleaved q-projection: all 4 output tiles accumulate in
            # parallel across 4 PSUM rings so each arriving (w_q, tgt)
            # chunk enables 4 matmuls instead of 1 -- the PE consumes the
            # DMA drip at the start instead of stalling per chunk
            qt_tags = ["acc", "st", "av", "acc"]
            qpmm = [ps.tile([P, 512], F32, tag=qt_tags[m], bufs=2,
                            name=f"qpmm{m}") for m in range(CHO)]
            for k in range(KO):
                for m in range(CHO):
                    nc.tensor.matmul(
                        qpmm[m][:], w_q[k][:, m * P:(m + 1) * P],
                        tgt_t[k][:, 0:512],
                        start=(k == 0), stop=(k == KO - 1))
            for m in range(CHO):
                nc.vector.tensor_scalar_add(
                    qT[:, m, 0:512], qpmm[m][:], qb_t[:, m:m + 1])

            for x in range(NSB):
                kt_group(0, x)
                v_group(x)

            prev = None
            for t in range(CHO):
                fill = []
                if t + 1 < CHO:
                    fill += [mk(kt_group, t + 1, x) for x in range(NSB)]
                else:
                    fill += [mk(qt_group, 0, 1)]
                pre = (lambda j: v_group(j) if j >= NSB else None) \
                    if t == 0 else None
                prev = attn_block(t, 0, fill, prev, pre)

            p0 = [mk(proj_group, m, 0) for m in range(KO)]
            p0_share = [0, 2, 3, 3]
            for t in range(CHO):
                fill = []
                if t + 1 < CHO:
                    fill += [mk(qt_group, t + 1, 1)]
                fill += [p0.pop(0) for _ in range(p0_share[t])]
                prev = attn_block(t, 1, fill, prev)
            # tail: drain avs, start 3 partial proj groups on the spare PSUM
            # rings to hide the final normalization latency, then finish
            prev[0]()
            pend = [(m, proj_start(m, 1, tag))
                    for m, tag in ((0, "st"), (1, "av"), (2, "st"),
                                   (3, "av"))]
            prev[1]()
            for m, pmm in pend:
                proj_finish(m, 1, pmm, use_scalar=True)
            for m in range(4, KO):
                proj_group(m, 1, use_scalar=True)
    nc.compile()
    return nc


_NC_CACHE: dict[int, "bacc.Bacc"] = {}


def kernel(tgt, src, src_padded_mask, q_w, q_b, kv_w, kv_b, proj_w, proj_b,
           _run_kwargs: dict | None = None):
    tgt = np.asarray(tgt, dtype=np.float32)
    src = np.asarray(src, dtype=np.float32)
    mask = np.asarray(src_padded_mask).astype(bool)
    q_w = np.asarray(q_w, dtype=np.float32)
    q_b = np.asarray(q_b, dtype=np.float32)
    kv_w = np.asarray(kv_w, dtype=np.float32)
    kv_b = np.asarray(kv_b, dtype=np.float32)
    proj_w = np.asarray(proj_w, dtype=np.float32)
    proj_b = np.asarray(proj_b, dtype=np.float32)

    # chunks of 128 src positions that are fully masked in EVERY batch can be
    # dropped at compile time; everything else is handled by the additive mask
    mchunk = mask.reshape(B, NS // P, P)
    dead = mchunk.all(axis=2).all(axis=0)            # [16]
    kept = [c for c in range(NS // P) if not dead[c]]
    if not kept:
        kept = [0]
    nk = len(kept)

    nc = _NC_CACHE.get(nk)
    if nc is None:
        nc = _build_nc(nk)
        _NC_CACHE[nk] = nc

    maskadd = np.where(mask, np.float32(NEG), np.float32(0.0)).astype(np.float32)
    bf = ml_dtypes.bfloat16

    in_maps = []
    for c in range(2 * B):
        b, g = c // 2, c % 2
        gs, ge = g * CH, (g + 1) * CH
        keep_pos = np.concatenate([np.arange(c * P, (c + 1) * P) for c in kept])
        in_maps.append({
            "tgtT": np.ascontiguousarray(tgt[b].T).astype(bf),
            "srcT": np.ascontiguousarray(src[b].T[:, keep_pos]).astype(bf),
            "qwT": np.ascontiguousarray(q_w[gs:ge].T).astype(bf),
            "kwT": np.ascontiguousarray(kv_w[gs:ge].T).astype(bf),
            "vwT": np.ascontiguousarray(kv_w[D + gs:D + ge].T).astype(bf),
            "pwT": np.ascontiguousarray(proj_w[:, gs:ge].T).astype(bf),
            "qb": q_b[gs:ge].copy(),
            "kb": kv_b[gs:ge].copy(),
            "vb": kv_b[D + gs:D + ge].copy(),
            "maskT": np.ascontiguousarray(maskadd[b][keep_pos].reshape(nk, P).T),
        })

    res = run_bass_kernel_spmd(nc, in_maps, list(range(2 * B)),
                               **(_run_kwargs or {}))
    if _run_kwargs:
        kernel.last_result = res

    out = np.empty((B, NT, D), dtype=np.float32)
    for b in range(B):
        part = (res.results[2 * b]["outT"].astype(np.float32)
                + res.results[2 * b + 1]["outT"].astype(np.float32))
        out[b] = part.T + proj_b
    return out
